# revision 9
# baseline (speedup 1.0000x reference)
"""Embedding-lookup MF model kernel for Trainium2 (8 NeuronCores).

reference math (B = 16384, D = 64):
    u   = user_table[x[:, 0]]          # [B, D]
    v   = item_table[x[:, 1]]          # [B, D]
    out = sigmoid(sum(u * v, -1))      # [B]

Strategy: data-parallel across the batch. Each of the 8 cores handles 2048
batch rows. The two tables are concatenated host-side into one [U+I, D]
table (user ids produced by the reference's randint fill are < 100000, so
only that prefix of the 1M-row user table is ever referenced; we upload a
prefix sized to the actual max id).

The TRN2 indirect-DMA primitive consumes exactly ONE index per destination
partition and fills that partition's dest extent contiguously from
table[idx[p]] (verified on HW). So each gather instruction moves 128 rows:
dest [128, 64] slice, offsets [128, 1]. 2048 u-rows + 2048 v-rows per core
= 32 gather instructions, pipelined with the DVE mul + segmented-reduce and
ACT sigmoid per chunk.

Layout per core (P=128 partitions, NBLK=16 blocks):
    batch row  b = n*128 + p   lives at  partition p, block n
    idx  SBUF tile [128, 32] int32: col n       = u-id of block n
                                    col 16 + n  = (u_rows + v-id) of block n
    gather tile tg [128, 2048] f32: u rows at cols [0,1024), v at [1024,2048)
"""

import os

# A previously crashed process can leave the NeuronCores wedged
# (NRT_EXEC_UNIT_UNRECOVERABLE on the next run); requesting a core reset at
# runtime init is harmless otherwise and self-heals that state.
os.environ.setdefault("NEURON_RT_RESET_CORES", "1")

import numpy as np

import concourse.bass as bass
import concourse.mybir as mybir
import concourse.tile as tile
from concourse import bacc
from concourse.bass_utils import run_bass_kernel_spmd

N_CORES = 8
P = 128
D = 64
B = 16384
BPC = B // N_CORES  # 2048 batch rows per core
NBLK = BPC // P  # 16 column blocks of 128 batch rows
# Tapered chunking: desc-gen for all 32 gathers is serial on the Q7, so only
# the LAST chunk's DMA-receipt + mul/reduce/sigmoid/store chain is exposed at
# the tail. Keep the last chunk minimal.
CHUNK_BLOCKS = [5, 5, 5, 1]

_programs: dict = {}


def _build(cat_rows: int):
    """Build the single-core program (run SPMD on 8 cores)."""
    nc = bacc.Bacc(
        "TRN2",
        target_bir_lowering=False,
        debug=False,
        detect_race_conditions=False,
    )
    idx = nc.dram_tensor("idx", [P, 2 * NBLK], mybir.dt.int32, kind="ExternalInput")
    tbl = nc.dram_tensor("tbl", [cat_rows, D], mybir.dt.float32, kind="ExternalInput")
    out = nc.dram_tensor("out", [P, NBLK], mybir.dt.float32, kind="ExternalOutput")

    # Load the indices in a raw pre-Tile block: the idx DMA then issues as
    # soon as the sync queue is up, and the TileContext start barrier (which
    # sync only reaches after wait_ge on the DMA) guarantees every engine sees
    # the data before any Tile instruction runs.
    t_idx = nc.alloc_sbuf_tensor("t_idx", [P, 2 * NBLK], mybir.dt.int32)
    idx_sem = nc.alloc_semaphore("idx_sem")
    with nc.Block() as blk:

        @blk.sync
        def _(sync):
            sync.dma_start(out=t_idx[:], in_=idx[:]).then_inc(idx_sem, 16)
            sync.wait_ge(idx_sem, 16)

    with tile.TileContext(nc) as tc:
        with (
            tc.tile_pool(name="io", bufs=1) as io_pool,
            tc.tile_pool(name="prod", bufs=2) as prod_pool,
        ):
            tg = io_pool.tile([P, 2 * NBLK * D], mybir.dt.float32)
            t_res = io_pool.tile([P, NBLK], mybir.dt.float32)
            # zero bias tile for the sigmoid activation: avoids the const-AP
            # DMA the framework would otherwise emit ahead of the idx load
            t_bias = io_pool.tile([P, 1], mybir.dt.float32)
            nc.vector.memset(t_bias[:], 0.0)
            b0 = 0
            for nb in CHUNK_BLOCKS:
                b1 = b0 + nb
                # gather this chunk's u blocks and v blocks, one row per
                # partition per instruction
                for j in list(range(b0, b1)) + list(range(NBLK + b0, NBLK + b1)):
                    nc.gpsimd.indirect_dma_start(
                        out=tg[:, j * D : (j + 1) * D],
                        out_offset=None,
                        in_=tbl[:],
                        in_offset=bass.IndirectOffsetOnAxis(
                            ap=t_idx[:, j : j + 1], axis=0
                        ),
                    )
                w = prod_pool.tile([P, nb * D], mybir.dt.float32, tag="w")
                nc.vector.tensor_mul(
                    out=w[:],
                    in0=tg[:, b0 * D : b1 * D],
                    in1=tg[:, (NBLK + b0) * D : (NBLK + b1) * D],
                )
                rs = t_res[:, b0:b1]
                nc.vector.reduce_sum(
                    out=rs,
                    in_=w[:].rearrange("p (n d) -> p n d", d=D),
                    axis=mybir.AxisListType.X,
                )
                nc.scalar.activation(
                    out=rs,
                    in_=rs,
                    func=mybir.ActivationFunctionType.Sigmoid,
                    bias=t_bias[:],
                )
                # store each chunk as soon as its sigmoid lands; only the last
                # (1-block) store sits on the critical tail
                nc.sync.dma_start(out=out[:, b0:b1], in_=t_res[:, b0:b1])
                b0 = b1
    nc.compile()
    return nc


def _get_program(cat_rows: int):
    if cat_rows not in _programs:
        _programs[cat_rows] = _build(cat_rows)
    return _programs[cat_rows]


def _prep_idx(xs: np.ndarray, u_rows: int) -> np.ndarray:
    """[BPC, 2] int32 -> [128, 32] idx tile (u cols then offset v cols)."""
    iu = xs[:, 0].reshape(NBLK, P).T  # [P, NBLK]
    iv = xs[:, 1].reshape(NBLK, P).T + u_rows
    return np.ascontiguousarray(np.concatenate([iu, iv], axis=1), dtype=np.int32)


def _run(x, user_table, item_table, **run_kwargs):
    x = np.asarray(x)
    ut = np.asarray(user_table, dtype=np.float32)
    it = np.asarray(item_table, dtype=np.float32)
    assert x.shape == (B, 2), x.shape
    xi = x.astype(np.int32)
    # user ids from the reference's randint fill are < 100000; upload only
    # the prefix of the user table that can actually be referenced.
    u_rows = min(ut.shape[0], max(100_000, int(xi[:, 0].max()) + 1))
    cat = np.ascontiguousarray(np.concatenate([ut[:u_rows], it], axis=0))
    nc = _get_program(cat.shape[0])
    in_maps = []
    for k in range(N_CORES):
        xs = xi[k * BPC : (k + 1) * BPC]
        in_maps.append({"idx": _prep_idx(xs, u_rows), "tbl": cat})
    res = run_bass_kernel_spmd(nc, in_maps, list(range(N_CORES)), **run_kwargs)
    out = np.empty(B, np.float32)
    for k in range(N_CORES):
        out[k * BPC : (k + 1) * BPC] = res.results[k]["out"].T.ravel()
    return out, res


def kernel(x, user_table, item_table):
    out, _ = _run(x, user_table, item_table)
    return out
